# revision 10
# baseline (speedup 1.0000x reference)
"""Trainium2 Bass kernel for nn_ChebySemi (Chebyshev semi-iteration with
per-sample 3x3 stencil conv + power iteration), data-parallel over 8 cores.

Algorithm per sample (matches reference.py):
  power: 20x { y = conv3x3(pad01(u)); m = max|y|; u = y/m }   -> m = m_20
  taus[k] = (1/m) * 2/(1.5 + 0.5*root_k)
  cheb:  15x { x += tau_k*(f - conv3x3(pad01(x))) }
where pad01 pads top/left with 0 and bottom/right with 1 (affine operator).

Implementation notes (v2):
  - conv = banded matmuls (f32r): per 126-row chunk, 3 col-shifted matmuls
    accumulate in PSUM; 4 main chunks (2 PSUM pairs) + 10-row tail.
  - The tail chunk stores the bottom PAD row at partition 0 (q=0 -> P513,
    q=1..9 -> P504..512) so every compute op uses partition base 0.
  - power phase runs UN-normalized: U_t = d_t * u_t with the pad cells also
    holding d_t, which keeps the affine pad term consistent.  Since
    max|u_t| = 1, d_t = max|psum_t| -- exactly the per-iteration reduction
    (DVE abs-max + gpsimd partition_all_reduce), written into the pad
    col/row cells each iteration.  PSUM->SBUF copies are plain ACT copies,
    so PSUM banks recycle fast and the PE never stalls on the max chain.
    One true rescale at t=10 keeps values in fp32 range.
    m = MX20/MX19 per sample.
  - cheb: f is folded into PSUM with an extra -I (bf16) matmul per chunk,
    so the update is a single DVE scalar_tensor_tensor per PSUM group:
    x += (-tau) * (Ax - f).
  - halo rows between chunks refreshed by 4 SBUF-SBUF DMAs per sample/iter,
    copying cols [0:513) only -- the pad col 513 of every row (incl. halo
    rows) is maintained by the pad-col write, so halos depend only on the
    ACT copies, not on the max chain.
"""
import numpy as np
import ml_dtypes

B = 64
NCORES = 8
SPC = B // NCORES          # samples per core
M = 512
PW = 514
CH = 5                     # row chunks (4 main + 1 tail)
MMO = 127                  # matmul M (psum partitions; col 0 of band zeroed)
TM = 10                    # tail rows / tail matmul M
SW = CH * PW               # per-sample free width in U/F
NPOW = 20
NCHEB = 15
ALPHA = 0.5
RESCALE_T = 10             # one mid-power rescale for fp32 range
ROOTS = np.cos(np.pi * (2 * np.arange(NCHEB) + 1) / (2 * NCHEB)).astype(np.float64)

_COMPILED = None


def _quant11(x):
    """Round fp32 to 11-bit mantissa (float32r input rounding)."""
    xi = np.ascontiguousarray(x, np.float32).view(np.uint32)
    shift = 23 - 11
    rb = np.uint32(1 << (shift - 1))
    mask = np.uint32(~((1 << shift) - 1) & 0xFFFFFFFF)
    return ((xi + rb) & mask).view(np.float32)


def _pad_layout(imgs, ones_pad):
    """imgs [N, 512, 512] -> [N, 128, 5, 514] chunk layout of padded P.

    Main chunks c<4 hold P rows [126c, 126c+128).  The tail chunk (c=4)
    is PERMUTED: partition 0 holds P row 513 (the bottom pad row),
    partitions 1..9 hold P rows 504..512."""
    n = imgs.shape[0]
    P = np.zeros((n, PW, PW), np.float32)
    P[:, 1:513, 1:513] = imgs
    if ones_pad:
        P[:, 513, :] = 1.0
        P[:, :, 513] = 1.0
    out = np.zeros((n, 128, CH, PW), np.float32)
    for c in range(4):
        out[:, :, c, :] = P[:, 126 * c:126 * c + 128, :]
    out[:, 0, 4, :] = P[:, 513, :]
    out[:, 1:10, 4, :] = P[:, 504:513, :]
    return out


def _bands(kern):
    """kern [N,3,3] -> [N, 128, 3*127] shifted bands, col p=0 zeroed."""
    n = kern.shape[0]
    S = np.zeros((n, 128, 3 * MMO), np.float32)
    for b in range(3):
        for a in range(3):
            # S[:, k, b*127+p] = K[a, b] where k = p - 1 + a, p in [1,127)
            p = np.arange(1, MMO)
            k = p - 1 + a
            ok = (k >= 0) & (k < 128)
            S[:, k[ok], b * MMO + p[ok]] = kern[:, a, b][:, None]
    return S


def _tail_bands(kern):
    """kern [N,3,3] -> [N, 10, 3*10] tail bands for the permuted tail.

    psum col p' (2..9) computes output row 502+p', reading P rows
    (502+p'+a) which live at tail partition q = p'-1+a, except P row 513
    which lives at q=0.  Cols p'=0,1 stay zero."""
    n = kern.shape[0]
    S = np.zeros((n, TM, 3 * TM), np.float32)
    for b in range(3):
        for a in range(3):
            for p in range(2, TM):
                row = 502 + p + a
                if row <= 512:
                    q = p - 1 + a
                elif row == 513:
                    q = 0
                else:
                    continue
                S[:, q, b * TM + p] = kern[:, a, b]
    return S


def _build_program():
    import concourse.bass as bass
    import concourse.tile as tile
    from concourse import mybir, bacc, bass_isa
    from contextlib import ExitStack

    F32 = mybir.dt.float32
    F32R = mybir.dt.float32r
    BF16 = mybir.dt.bfloat16
    AX = mybir.AxisListType
    OP = mybir.AluOpType

    nc = bacc.Bacc("TRN2", target_bir_lowering=False, debug=False)

    u0p_d = nc.dram_tensor("u0p", [128, SPC * SW], F32, kind="ExternalInput")
    xp_d = nc.dram_tensor("xp", [128, SPC * SW], F32, kind="ExternalInput")
    fp_d = nc.dram_tensor("fp", [128, SPC * SW], BF16, kind="ExternalInput")
    wb_d = nc.dram_tensor("wb", [128, SPC * 3 * MMO], F32, kind="ExternalInput")
    wt_d = nc.dram_tensor("wt", [128, SPC * 3 * TM], F32, kind="ExternalInput")
    negi_d = nc.dram_tensor("negi", [128, 128], BF16, kind="ExternalInput")
    c2b_d = nc.dram_tensor("c2b", [128, NCHEB], F32, kind="ExternalInput")
    c2bn_d = nc.dram_tensor("c2bn", [128, NCHEB], F32, kind="ExternalInput")
    out_d = nc.dram_tensor("out", [SPC * M, M], F32, kind="ExternalOutput")

    with tile.TileContext(nc) as tc, ExitStack() as ctx:
        sb = ctx.enter_context(tc.tile_pool(name="sb", bufs=1))
        ps = ctx.enter_context(tc.tile_pool(name="ps", bufs=3, space="PSUM"))
        p4p = ctx.enter_context(tc.tile_pool(name="p4p", bufs=2, space="PSUM"))

        U = sb.tile([128, SPC * SW], F32)
        Fm = sb.tile([128, SPC * SW], BF16)
        W = sb.tile([128, SPC * 3 * MMO], F32)
        WT = sb.tile([128, SPC * 3 * TM], F32)
        NEGI = sb.tile([128, 128], BF16)
        C2B = sb.tile([128, NCHEB], F32)
        C2BN = sb.tile([128, NCHEB], F32)
        TAU = sb.tile([128, SPC * NCHEB], F32)
        NTAU = sb.tile([128, SPC * NCHEB], F32)
        RA = sb.tile([128, SPC * 4], F32)     # per-sample partial abs-max cols
        RAC = sb.tile([128, SPC], F32)        # combined col per sample
        MXE = sb.tile([128, SPC], F32)        # per-iter max (bcast over parts)
        MX19 = sb.tile([128, SPC], F32)
        MX20 = sb.tile([128, SPC], F32)
        INVC = sb.tile([128, SPC], F32)       # reciprocal at rescale event
        IM20 = sb.tile([128, SPC], F32)
        INVM = sb.tile([128, SPC], F32)
        ONESW = sb.tile([128, 520], F32)

        # input loads: bands on the scalar HWDGE ring, per-sample U on the
        # sync ring (so conv(s=0) starts as soon as W/WT/U_0 land); Fm last.
        nc.scalar.dma_start(W[:].bitcast(F32R), wb_d.ap()[:, :].bitcast(F32R))
        nc.scalar.dma_start(WT[:].bitcast(F32R), wt_d.ap()[:, :].bitcast(F32R))
        for s in range(SPC):
            nc.sync.dma_start(U[:, s * SW:(s + 1) * SW].bitcast(F32R),
                              u0p_d.ap()[:, s * SW:(s + 1) * SW].bitcast(F32R))
        nc.scalar.dma_start(NEGI[:], negi_d.ap()[:, :])
        nc.scalar.dma_start(C2B[:], c2b_d.ap()[:, :])
        nc.scalar.dma_start(C2BN[:], c2bn_d.ap()[:, :])
        nc.scalar.dma_start(Fm[:], fp_d.ap()[:, :])
        nc.vector.memset(RA[:, :], 0.0)
        nc.vector.memset(ONESW[:, :], 1.0)

        def us(s):
            return U[:, s * SW:(s + 1) * SW].rearrange("p (c w) -> p c w", c=CH)

        def conv(s, with_f):
            """tail + 2 chunk-pair groups of banded matmuls; optionally
            accumulate -f via NEGI so psum = A x - f."""
            Us = us(s)
            Ws = W[:, s * 3 * MMO:(s + 1) * 3 * MMO]
            Wts = WT[:, s * 3 * TM:(s + 1) * 3 * TM]
            Fs = Fm[:, s * SW:(s + 1) * SW].rearrange("p (c w) -> p c w", c=CH)
            p4 = p4p.tile([128, 512], F32, tag="p4")
            for b in range(3):
                nc.tensor.matmul(
                    p4[0:TM, 0:512],
                    Wts[0:TM, b * TM:(b + 1) * TM].bitcast(F32R),
                    Us[0:TM, 4, b:b + 512].bitcast(F32R),
                    start=(b == 0), stop=(b == 2 and not with_f))
            if with_f:
                nc.tensor.matmul(
                    p4[0:TM, 0:512], NEGI[0:TM, 0:TM], Fs[0:TM, 4, 1:513],
                    start=False, stop=True)
            pm0 = ps.tile([128, 1024], F32, tag="pm")
            pm1 = ps.tile([128, 1024], F32, tag="pm")
            for g, pt in ((0, pm0), (1, pm1)):
                for ci in range(2):
                    c = 2 * g + ci
                    for b in range(3):
                        nc.tensor.matmul(
                            pt[0:MMO, ci * 512:(ci + 1) * 512],
                            Ws[:, b * MMO:(b + 1) * MMO].bitcast(F32R),
                            Us[0:128, c, b:b + 512].bitcast(F32R),
                            start=(b == 0), stop=(b == 2 and not with_f))
                    if with_f:
                        nc.tensor.matmul(
                            pt[0:MMO, ci * 512:(ci + 1) * 512],
                            NEGI[0:128, 0:MMO], Fs[0:128, c, 1:513],
                            start=False, stop=True)
            return pm0, pm1, p4

        def reduce_max(s, pm0, pm1, p4, target):
            """abs-max over the sample's psum -> target[:, s] bcast on parts."""
            c0 = 4 * s
            nc.vector.tensor_reduce(
                RA[0:MMO, c0:c0 + 1],
                pm0[0:MMO, :].rearrange("p (c w) -> p c w", c=2),
                axis=AX.XY, op=OP.max, apply_absolute_value=True)
            nc.vector.tensor_reduce(
                RA[0:MMO, c0 + 1:c0 + 2],
                pm1[0:MMO, :].rearrange("p (c w) -> p c w", c=2),
                axis=AX.XY, op=OP.max, apply_absolute_value=True)
            nc.vector.tensor_reduce(
                RA[0:TM, c0 + 2:c0 + 3], p4[0:TM, 0:512],
                axis=AX.X, op=OP.max, apply_absolute_value=True)
            nc.vector.tensor_reduce(
                RAC[0:128, s:s + 1], RA[0:128, c0:c0 + 3],
                axis=AX.X, op=OP.max)
            nc.gpsimd.partition_all_reduce(
                target[:, s:s + 1], RAC[:, s:s + 1], channels=128,
                reduce_op=bass_isa.ReduceOp.max)

        def copies(s, pm0, pm1, p4):
            # psum col p=0 is all-zero (zeroed band col), so base-0 writes are
            # safe: chunk0 partition 0 is the top pad row (stays 0); other
            # chunks' partition 0 are halo rows, refreshed by halo DMAs.
            Us = us(s)
            for g, pt in ((0, pm0), (1, pm1)):
                nc.scalar.copy(Us[0:MMO, 2 * g:2 * g + 2, 1:513].bitcast(F32R),
                               pt[0:MMO, :].rearrange("p (c w) -> p c w", c=2))
            nc.scalar.copy(Us[0:TM, 4, 1:513].bitcast(F32R), p4[0:TM, 0:512])

        def pad_writes(s, mx):
            Us = us(s)
            # right pad col (c,513) for all chunks, all partitions (incl.
            # halo rows -- halo DMAs skip col 513)
            nc.vector.tensor_scalar_mul(
                Us[0:128, 0:CH, 513:514].bitcast(F32R),
                ONESW[0:128, 0:CH].rearrange("p (c w) -> p c w", c=CH),
                mx[0:128, s:s + 1])
            # bottom pad row = tail partition 0, full 514 cols
            nc.scalar.mul(Us[0:1, 4, 0:514].bitcast(F32R),
                          ONESW[0:1, 0:514], mx[0:1, s:s + 1])

        def uv(p0, p1, sg, c0, c1):
            """U view [partitions p0:p1, samples 4sg..4sg+4, chunks c0:c1,
            cols 0:513] as one strided AP."""
            return U[p0:p1, :].rearrange(
                "p (s c w) -> p s c w", s=SPC, c=CH)[
                :, 4 * sg:4 * sg + 4, c0:c1, 0:513]

        def halos_batch(sg):
            # 8 DMAs refresh all halo rows of samples 4sg..4sg+3 (one per
            # direction x chunk, batched over the 4 samples); cols [0:513)
            # only (pad col 513 is kept by pad_writes).  DMA APs are
            # limited to 3 dims, hence per-chunk batching.
            for c in (1, 2, 3):
                # top halo row of chunk c <- row 126 of chunk c-1
                nc.sync.dma_start(uv(0, 1, sg, c, c + 1).bitcast(F32R),
                                  uv(126, 127, sg, c - 1, c).bitcast(F32R))
                # bottom halo row of chunk c-1 <- row 1 of chunk c
                nc.sync.dma_start(uv(127, 128, sg, c - 1, c).bitcast(F32R),
                                  uv(1, 2, sg, c, c + 1).bitcast(F32R))
            # tail: P504 at tail partition 1 <- row 126 of chunk 3;
            # bottom halo of chunk 3 <- P505 at tail partition 2
            nc.sync.dma_start(uv(1, 2, sg, 4, 5).bitcast(F32R),
                              uv(126, 127, sg, 3, 4).bitcast(F32R))
            nc.sync.dma_start(uv(127, 128, sg, 3, 4).bitcast(F32R),
                              uv(2, 3, sg, 4, 5).bitcast(F32R))

        # ---- power phase ----
        for it in range(1, NPOW + 1):
            for sg in range(2):
              for si in range(4):
                s = 4 * sg + si
                pm0, pm1, p4 = conv(s, with_f=False)
                target = MX20 if it == NPOW else (MX19 if it == NPOW - 1 else MXE)
                reduce_max(s, pm0, pm1, p4, target)
                if it < NPOW:
                    copies(s, pm0, pm1, p4)
                    pad_writes(s, target)
                    if it == RESCALE_T:
                        nc.vector.reciprocal(INVC[:, s:s + 1], MXE[:, s:s + 1])
                        Uf = U[:, s * SW:(s + 1) * SW]
                        nc.scalar.mul(Uf[:, 0:1028].bitcast(F32R),
                                      Uf[:, 0:1028], INVC[:, s:s + 1])
                        nc.vector.tensor_scalar_mul(
                            Uf[:, 1028:SW].bitcast(F32R), Uf[:, 1028:SW],
                            INVC[:, s:s + 1])
                else:
                    nc.scalar.dma_start(
                        U[:, s * SW:(s + 1) * SW].bitcast(F32R),
                        xp_d.ap()[:, s * SW:(s + 1) * SW].bitcast(F32R))
              if it < NPOW:
                halos_batch(sg)

        # ---- taus: tau_k(s) = (MX19/MX20)(s) * c2q[k], bcast on parts ----
        nc.vector.reciprocal(IM20[:, :], MX20[:, :])
        nc.vector.tensor_tensor(INVM[:, :], MX19[:, :], IM20[:, :], op=OP.mult)
        for s in range(SPC):
            nc.vector.tensor_scalar_mul(
                TAU[:, s * NCHEB:(s + 1) * NCHEB], C2B[:, :], INVM[:, s:s + 1])
            nc.vector.tensor_scalar_mul(
                NTAU[:, s * NCHEB:(s + 1) * NCHEB], C2BN[:, :], INVM[:, s:s + 1])

        # ---- cheb phase:  x += (-tau) * (Ax - f) ----
        for k in range(NCHEB):
            for sg in range(2):
              for si in range(4):
                s = 4 * sg + si
                pm0, pm1, p4 = conv(s, with_f=True)
                Us = us(s)
                ntc = NTAU[:, s * NCHEB + k:s * NCHEB + k + 1]
                for g, pt in ((0, pm0), (1, pm1)):
                    nc.vector.scalar_tensor_tensor(
                        Us[0:MMO, 2 * g:2 * g + 2, 1:513].bitcast(F32R),
                        pt[0:MMO, :].rearrange("p (c w) -> p c w", c=2),
                        ntc[0:MMO, :],
                        Us[0:MMO, 2 * g:2 * g + 2, 1:513],
                        op0=OP.mult, op1=OP.add)
                nc.vector.scalar_tensor_tensor(
                    Us[0:TM, 4, 1:513].bitcast(F32R), p4[0:TM, 0:512],
                    ntc[0:TM, :], Us[0:TM, 4, 1:513], op0=OP.mult, op1=OP.add)
                if k == NCHEB - 1:
                    o = out_d.ap()[s * M:(s + 1) * M, :]
                    nc.sync.dma_start(
                        o[0:504, :].rearrange("(c p) w -> p c w", p=126),
                        Us[1:MMO, 0:4, 1:513])
                    nc.scalar.dma_start(o[504:512, :], Us[2:TM, 4, 1:513])
              if k < NCHEB - 1:
                halos_batch(sg)

    nc.compile()
    return nc


def _prep_core_inputs(x, f, kernelA, u0):
    """Full [64,...] inputs -> list of 8 per-core input dicts."""
    x = np.asarray(x, np.float32).reshape(B, M, M)
    f = np.asarray(f, np.float32).reshape(B, M, M)
    kern = np.asarray(kernelA, np.float32).reshape(B, 3, 3)
    u0 = np.asarray(u0, np.float32).reshape(B, M, M)

    u0L = _quant11(_pad_layout(u0, True))     # [B,128,CH,PW]
    xL = _quant11(_pad_layout(x, True))
    fL = _pad_layout(f, False).astype(ml_dtypes.bfloat16)
    wbL = _quant11(_bands(kern))              # [B,128,381]
    wtL = np.zeros((B, 128, 3 * TM), np.float32)
    wtL[:, 0:TM, :] = _quant11(_tail_bands(kern))

    c2q = (2.0 / (1.5 + 0.5 * ROOTS)).astype(np.float32)
    c2bT = np.broadcast_to(c2q, (128, NCHEB)).copy()
    c2bnT = (-c2bT).copy()
    negi = (-np.eye(128)).astype(ml_dtypes.bfloat16)

    in_maps = []
    for c in range(NCORES):
        sl = slice(c * SPC, (c + 1) * SPC)
        in_maps.append({
            "u0p": u0L[sl].transpose(1, 0, 2, 3).reshape(128, SPC * SW).copy(),
            "xp": xL[sl].transpose(1, 0, 2, 3).reshape(128, SPC * SW).copy(),
            "fp": fL[sl].transpose(1, 0, 2, 3).reshape(128, SPC * SW).copy(),
            "wb": wbL[sl].transpose(1, 0, 2).reshape(128, SPC * 3 * MMO).copy(),
            "wt": wtL[sl].transpose(1, 0, 2).reshape(128, SPC * 3 * TM).copy(),
            "negi": negi,
            "c2b": c2bT,
            "c2bn": c2bnT,
        })
    return in_maps


def kernel(x, f, kernelA, u0):
    global _COMPILED
    from concourse import bass_utils

    if _COMPILED is None:
        _COMPILED = _build_program()
    nc = _COMPILED

    in_maps = _prep_core_inputs(x, f, kernelA, u0)
    res = bass_utils.run_bass_kernel_spmd(nc, in_maps, core_ids=list(range(NCORES)))
    out = np.stack([res.results[c]["out"] for c in range(NCORES)])  # [8, SPC*M, M]
    return out.reshape(B, 1, M, M).astype(np.float32)


# revision 11
# speedup vs baseline: 1.0383x; 1.0383x over previous
"""Trainium2 Bass kernel for nn_ChebySemi (Chebyshev semi-iteration with
per-sample 3x3 stencil conv + power iteration), data-parallel over 8 cores.

Algorithm per sample (matches reference.py):
  power: 20x { y = conv3x3(pad01(u)); m = max|y|; u = y/m }   -> m = m_20
  taus[k] = (1/m) * 2/(1.5 + 0.5*root_k)
  cheb:  15x { x += tau_k*(f - conv3x3(pad01(x))) }
where pad01 pads top/left with 0 and bottom/right with 1 (affine operator).

Implementation notes (v2):
  - conv = banded matmuls (f32r): per 126-row chunk, 3 col-shifted matmuls
    accumulate in PSUM; 4 main chunks (2 PSUM pairs) + 10-row tail.
  - The tail chunk stores the bottom PAD row at partition 0 (q=0 -> P513,
    q=1..9 -> P504..512) so every compute op uses partition base 0.
  - power phase runs UN-normalized: U_t = d_t * u_t with the pad cells also
    holding d_t, which keeps the affine pad term consistent.  Since
    max|u_t| = 1, d_t = max|psum_t| -- exactly the per-iteration reduction
    (DVE abs-max + gpsimd partition_all_reduce), written into the pad
    col/row cells each iteration.  PSUM->SBUF copies are plain ACT copies,
    so PSUM banks recycle fast and the PE never stalls on the max chain.
    One true rescale at t=10 keeps values in fp32 range.
    m = MX20/MX19 per sample.
  - cheb: f is folded into PSUM with an extra -I (bf16) matmul per chunk,
    so the update is a single DVE scalar_tensor_tensor per PSUM group:
    x += (-tau) * (Ax - f).
  - halo rows between chunks refreshed by 4 SBUF-SBUF DMAs per sample/iter.
"""
import numpy as np
import ml_dtypes

B = 64
NCORES = 8
SPC = B // NCORES          # samples per core
M = 512
PW = 514
CH = 5                     # row chunks (4 main + 1 tail)
MMO = 127                  # matmul M (psum partitions; col 0 of band zeroed)
TM = 10                    # tail rows / tail matmul M
SW = CH * PW               # per-sample free width in U/F
NPOW = 20
NCHEB = 15
ALPHA = 0.5
RESCALE_T = 10             # one mid-power rescale for fp32 range
ROOTS = np.cos(np.pi * (2 * np.arange(NCHEB) + 1) / (2 * NCHEB)).astype(np.float64)

_COMPILED = None


def _quant11(x):
    """Round fp32 to 11-bit mantissa (float32r input rounding)."""
    xi = np.ascontiguousarray(x, np.float32).view(np.uint32)
    shift = 23 - 11
    rb = np.uint32(1 << (shift - 1))
    mask = np.uint32(~((1 << shift) - 1) & 0xFFFFFFFF)
    return ((xi + rb) & mask).view(np.float32)


def _pad_layout(imgs, ones_pad):
    """imgs [N, 512, 512] -> [N, 128, 5, 514] chunk layout of padded P.

    Main chunks c<4 hold P rows [126c, 126c+128).  The tail chunk (c=4)
    is PERMUTED: partition 0 holds P row 513 (the bottom pad row),
    partitions 1..9 hold P rows 504..512."""
    n = imgs.shape[0]
    P = np.zeros((n, PW, PW), np.float32)
    P[:, 1:513, 1:513] = imgs
    if ones_pad:
        P[:, 513, :] = 1.0
        P[:, :, 513] = 1.0
    out = np.zeros((n, 128, CH, PW), np.float32)
    for c in range(4):
        out[:, :, c, :] = P[:, 126 * c:126 * c + 128, :]
    out[:, 0, 4, :] = P[:, 513, :]
    out[:, 1:10, 4, :] = P[:, 504:513, :]
    return out


def _bands(kern):
    """kern [N,3,3] -> [N, 128, 3*127] shifted bands, col p=0 zeroed."""
    n = kern.shape[0]
    S = np.zeros((n, 128, 3 * MMO), np.float32)
    for b in range(3):
        for a in range(3):
            # S[:, k, b*127+p] = K[a, b] where k = p - 1 + a, p in [1,127)
            p = np.arange(1, MMO)
            k = p - 1 + a
            ok = (k >= 0) & (k < 128)
            S[:, k[ok], b * MMO + p[ok]] = kern[:, a, b][:, None]
    return S


def _tail_bands(kern):
    """kern [N,3,3] -> [N, 10, 3*10] tail bands for the permuted tail.

    psum col p' (2..9) computes output row 502+p', reading P rows
    (502+p'+a) which live at tail partition q = p'-1+a, except P row 513
    which lives at q=0.  Cols p'=0,1 stay zero."""
    n = kern.shape[0]
    S = np.zeros((n, TM, 3 * TM), np.float32)
    for b in range(3):
        for a in range(3):
            for p in range(2, TM):
                row = 502 + p + a
                if row <= 512:
                    q = p - 1 + a
                elif row == 513:
                    q = 0
                else:
                    continue
                S[:, q, b * TM + p] = kern[:, a, b]
    return S


def _build_program():
    import concourse.bass as bass
    import concourse.tile as tile
    from concourse import mybir, bacc, bass_isa
    from contextlib import ExitStack

    F32 = mybir.dt.float32
    F32R = mybir.dt.float32r
    BF16 = mybir.dt.bfloat16
    AX = mybir.AxisListType
    OP = mybir.AluOpType

    nc = bacc.Bacc("TRN2", target_bir_lowering=False, debug=False)

    u0p_d = nc.dram_tensor("u0p", [128, SPC * SW], F32, kind="ExternalInput")
    xp_d = nc.dram_tensor("xp", [128, SPC * SW], F32, kind="ExternalInput")
    fp_d = nc.dram_tensor("fp", [128, SPC * SW], BF16, kind="ExternalInput")
    wb_d = nc.dram_tensor("wb", [128, SPC * 3 * MMO], F32, kind="ExternalInput")
    wt_d = nc.dram_tensor("wt", [128, SPC * 3 * TM], F32, kind="ExternalInput")
    negi_d = nc.dram_tensor("negi", [128, 128], BF16, kind="ExternalInput")
    c2b_d = nc.dram_tensor("c2b", [128, NCHEB], F32, kind="ExternalInput")
    c2bn_d = nc.dram_tensor("c2bn", [128, NCHEB], F32, kind="ExternalInput")
    out_d = nc.dram_tensor("out", [SPC * M, M], F32, kind="ExternalOutput")

    with tile.TileContext(nc) as tc, ExitStack() as ctx:
        sb = ctx.enter_context(tc.tile_pool(name="sb", bufs=1))
        ps = ctx.enter_context(tc.tile_pool(name="ps", bufs=3, space="PSUM"))
        p4p = ctx.enter_context(tc.tile_pool(name="p4p", bufs=2, space="PSUM"))

        U = sb.tile([128, SPC * SW], F32)
        Fm = sb.tile([128, SPC * SW], BF16)
        W = sb.tile([128, SPC * 3 * MMO], F32)
        WT = sb.tile([128, SPC * 3 * TM], F32)
        NEGI = sb.tile([128, 128], BF16)
        C2B = sb.tile([128, NCHEB], F32)
        C2BN = sb.tile([128, NCHEB], F32)
        TAU = sb.tile([128, SPC * NCHEB], F32)
        NTAU = sb.tile([128, SPC * NCHEB], F32)
        RA = sb.tile([128, SPC * 4], F32)     # per-sample partial abs-max cols
        RAC = sb.tile([128, SPC], F32)        # combined col per sample
        MXE = sb.tile([128, SPC], F32)        # per-iter max (bcast over parts)
        MX19 = sb.tile([128, SPC], F32)
        MX20 = sb.tile([128, SPC], F32)
        INVC = sb.tile([128, SPC], F32)       # reciprocal at rescale event
        IM20 = sb.tile([128, SPC], F32)
        INVM = sb.tile([128, SPC], F32)
        ONESW = sb.tile([128, 520], F32)

        # input loads, all on the sync HWDGE ring: bands first, then U per
        # sample (so conv(s) can start as soon as W/WT/U_s land), Fm last.
        nc.sync.dma_start(W[:].bitcast(F32R), wb_d.ap()[:, :].bitcast(F32R))
        nc.sync.dma_start(WT[:].bitcast(F32R), wt_d.ap()[:, :].bitcast(F32R))
        for s in range(SPC):
            nc.sync.dma_start(U[:, s * SW:(s + 1) * SW].bitcast(F32R),
                              u0p_d.ap()[:, s * SW:(s + 1) * SW].bitcast(F32R))
        nc.sync.dma_start(NEGI[:], negi_d.ap()[:, :])
        nc.sync.dma_start(C2B[:], c2b_d.ap()[:, :])
        nc.sync.dma_start(C2BN[:], c2bn_d.ap()[:, :])
        nc.sync.dma_start(Fm[:], fp_d.ap()[:, :])
        nc.vector.memset(RA[:, :], 0.0)
        nc.vector.memset(ONESW[:, :], 1.0)

        def us(s):
            return U[:, s * SW:(s + 1) * SW].rearrange("p (c w) -> p c w", c=CH)

        def conv(s, with_f):
            """tail + 2 chunk-pair groups of banded matmuls; optionally
            accumulate -f via NEGI so psum = A x - f."""
            Us = us(s)
            Ws = W[:, s * 3 * MMO:(s + 1) * 3 * MMO]
            Wts = WT[:, s * 3 * TM:(s + 1) * 3 * TM]
            Fs = Fm[:, s * SW:(s + 1) * SW].rearrange("p (c w) -> p c w", c=CH)
            p4 = p4p.tile([128, 512], F32, tag="p4")
            for b in range(3):
                nc.tensor.matmul(
                    p4[0:TM, 0:512],
                    Wts[0:TM, b * TM:(b + 1) * TM].bitcast(F32R),
                    Us[0:TM, 4, b:b + 512].bitcast(F32R),
                    start=(b == 0), stop=(b == 2 and not with_f))
            if with_f:
                nc.tensor.matmul(
                    p4[0:TM, 0:512], NEGI[0:TM, 0:TM], Fs[0:TM, 4, 1:513],
                    start=False, stop=True)
            pm0 = ps.tile([128, 1024], F32, tag="pm")
            pm1 = ps.tile([128, 1024], F32, tag="pm")
            for g, pt in ((0, pm0), (1, pm1)):
                for ci in range(2):
                    c = 2 * g + ci
                    for b in range(3):
                        nc.tensor.matmul(
                            pt[0:MMO, ci * 512:(ci + 1) * 512],
                            Ws[:, b * MMO:(b + 1) * MMO].bitcast(F32R),
                            Us[0:128, c, b:b + 512].bitcast(F32R),
                            start=(b == 0), stop=(b == 2 and not with_f))
                    if with_f:
                        nc.tensor.matmul(
                            pt[0:MMO, ci * 512:(ci + 1) * 512],
                            NEGI[0:128, 0:MMO], Fs[0:128, c, 1:513],
                            start=False, stop=True)
            return pm0, pm1, p4

        def reduce_max(s, pm0, pm1, p4, target):
            """abs-max over the sample's psum -> target[:, s] bcast on parts."""
            c0 = 4 * s
            nc.vector.tensor_reduce(
                RA[0:MMO, c0:c0 + 1],
                pm0[0:MMO, :].rearrange("p (c w) -> p c w", c=2),
                axis=AX.XY, op=OP.max, apply_absolute_value=True)
            nc.vector.tensor_reduce(
                RA[0:MMO, c0 + 1:c0 + 2],
                pm1[0:MMO, :].rearrange("p (c w) -> p c w", c=2),
                axis=AX.XY, op=OP.max, apply_absolute_value=True)
            nc.vector.tensor_reduce(
                RA[0:TM, c0 + 2:c0 + 3], p4[0:TM, 0:512],
                axis=AX.X, op=OP.max, apply_absolute_value=True)
            nc.vector.tensor_reduce(
                RAC[0:128, s:s + 1], RA[0:128, c0:c0 + 3],
                axis=AX.X, op=OP.max)
            nc.gpsimd.partition_all_reduce(
                target[:, s:s + 1], RAC[:, s:s + 1], channels=128,
                reduce_op=bass_isa.ReduceOp.max)

        def copies(s, pm0, pm1, p4):
            # psum col p=0 is all-zero (zeroed band col), so base-0 writes are
            # safe: chunk0 partition 0 is the top pad row (stays 0); other
            # chunks' partition 0 are halo rows, refreshed by halo DMAs.
            Us = us(s)
            for g, pt in ((0, pm0), (1, pm1)):
                nc.scalar.copy(Us[0:MMO, 2 * g:2 * g + 2, 1:513].bitcast(F32R),
                               pt[0:MMO, :].rearrange("p (c w) -> p c w", c=2))
            nc.scalar.copy(Us[0:TM, 4, 1:513].bitcast(F32R), p4[0:TM, 0:512])

        def pad_writes(s, mx):
            Us = us(s)
            # right pad col (c,513) for all chunks, all partitions (incl.
            # halo rows -- halo DMAs skip col 513)
            nc.vector.tensor_scalar_mul(
                Us[0:128, 0:CH, 513:514].bitcast(F32R),
                ONESW[0:128, 0:CH].rearrange("p (c w) -> p c w", c=CH),
                mx[0:128, s:s + 1])
            # bottom pad row = tail partition 0, full 514 cols
            nc.scalar.mul(Us[0:1, 4, 0:514].bitcast(F32R),
                          ONESW[0:1, 0:514], mx[0:1, s:s + 1])

        def halos(s):
            Us = us(s)
            # top halo rows (P row 126c at partition 0 of chunks 1..3;
            # P row 504 at tail partition 1)
            nc.sync.dma_start(Us[0:1, 1:4, :].bitcast(F32R),
                              Us[126:127, 0:3, :].bitcast(F32R))
            nc.sync.dma_start(Us[1:2, 4:5, :].bitcast(F32R),
                              Us[126:127, 3:4, :].bitcast(F32R))
            # bottom halo rows (P row 126c+127 at partition 127 of chunks
            # 0..2; P row 505 for chunk 3 comes from tail partition 2)
            nc.sync.dma_start(Us[127:128, 0:3, :].bitcast(F32R),
                              Us[1:2, 1:4, :].bitcast(F32R))
            nc.sync.dma_start(Us[127:128, 3:4, :].bitcast(F32R),
                              Us[2:3, 4:5, :].bitcast(F32R))

        # ---- power phase ----
        for it in range(1, NPOW + 1):
            for s in range(SPC):
                pm0, pm1, p4 = conv(s, with_f=False)
                target = MX20 if it == NPOW else (MX19 if it == NPOW - 1 else MXE)
                reduce_max(s, pm0, pm1, p4, target)
                if it < NPOW:
                    copies(s, pm0, pm1, p4)
                    pad_writes(s, target)
                    if it == RESCALE_T:
                        nc.vector.reciprocal(INVC[:, s:s + 1], MXE[:, s:s + 1])
                        Uf = U[:, s * SW:(s + 1) * SW]
                        nc.scalar.mul(Uf[:, 0:1028].bitcast(F32R),
                                      Uf[:, 0:1028], INVC[:, s:s + 1])
                        nc.vector.tensor_scalar_mul(
                            Uf[:, 1028:SW].bitcast(F32R), Uf[:, 1028:SW],
                            INVC[:, s:s + 1])
                    halos(s)
                else:
                    nc.sync.dma_start(
                        U[:, s * SW:(s + 1) * SW].bitcast(F32R),
                        xp_d.ap()[:, s * SW:(s + 1) * SW].bitcast(F32R))

        # ---- taus: tau_k(s) = (MX19/MX20)(s) * c2q[k], bcast on parts ----
        nc.vector.reciprocal(IM20[:, :], MX20[:, :])
        nc.vector.tensor_tensor(INVM[:, :], MX19[:, :], IM20[:, :], op=OP.mult)
        for s in range(SPC):
            nc.vector.tensor_scalar_mul(
                TAU[:, s * NCHEB:(s + 1) * NCHEB], C2B[:, :], INVM[:, s:s + 1])
            nc.vector.tensor_scalar_mul(
                NTAU[:, s * NCHEB:(s + 1) * NCHEB], C2BN[:, :], INVM[:, s:s + 1])

        # ---- cheb phase:  x += (-tau) * (Ax - f) ----
        for k in range(NCHEB):
            for s in range(SPC):
                pm0, pm1, p4 = conv(s, with_f=True)
                Us = us(s)
                ntc = NTAU[:, s * NCHEB + k:s * NCHEB + k + 1]
                for g, pt in ((0, pm0), (1, pm1)):
                    nc.vector.scalar_tensor_tensor(
                        Us[0:MMO, 2 * g:2 * g + 2, 1:513].bitcast(F32R),
                        pt[0:MMO, :].rearrange("p (c w) -> p c w", c=2),
                        ntc[0:MMO, :],
                        Us[0:MMO, 2 * g:2 * g + 2, 1:513],
                        op0=OP.mult, op1=OP.add)
                nc.vector.scalar_tensor_tensor(
                    Us[0:TM, 4, 1:513].bitcast(F32R), p4[0:TM, 0:512],
                    ntc[0:TM, :], Us[0:TM, 4, 1:513], op0=OP.mult, op1=OP.add)
                if k < NCHEB - 1:
                    halos(s)
                else:
                    o = out_d.ap()[s * M:(s + 1) * M, :]
                    nc.sync.dma_start(
                        o[0:504, :].rearrange("(c p) w -> p c w", p=126),
                        Us[1:MMO, 0:4, 1:513])
                    nc.sync.dma_start(o[504:512, :], Us[2:TM, 4, 1:513])

    nc.compile()
    return nc


def _prep_core_inputs(x, f, kernelA, u0):
    """Full [64,...] inputs -> list of 8 per-core input dicts."""
    x = np.asarray(x, np.float32).reshape(B, M, M)
    f = np.asarray(f, np.float32).reshape(B, M, M)
    kern = np.asarray(kernelA, np.float32).reshape(B, 3, 3)
    u0 = np.asarray(u0, np.float32).reshape(B, M, M)

    u0L = _quant11(_pad_layout(u0, True))     # [B,128,CH,PW]
    xL = _quant11(_pad_layout(x, True))
    fL = _pad_layout(f, False).astype(ml_dtypes.bfloat16)
    wbL = _quant11(_bands(kern))              # [B,128,381]
    wtL = np.zeros((B, 128, 3 * TM), np.float32)
    wtL[:, 0:TM, :] = _quant11(_tail_bands(kern))

    c2q = (2.0 / (1.5 + 0.5 * ROOTS)).astype(np.float32)
    c2bT = np.broadcast_to(c2q, (128, NCHEB)).copy()
    c2bnT = (-c2bT).copy()
    negi = (-np.eye(128)).astype(ml_dtypes.bfloat16)

    in_maps = []
    for c in range(NCORES):
        sl = slice(c * SPC, (c + 1) * SPC)
        in_maps.append({
            "u0p": u0L[sl].transpose(1, 0, 2, 3).reshape(128, SPC * SW).copy(),
            "xp": xL[sl].transpose(1, 0, 2, 3).reshape(128, SPC * SW).copy(),
            "fp": fL[sl].transpose(1, 0, 2, 3).reshape(128, SPC * SW).copy(),
            "wb": wbL[sl].transpose(1, 0, 2).reshape(128, SPC * 3 * MMO).copy(),
            "wt": wtL[sl].transpose(1, 0, 2).reshape(128, SPC * 3 * TM).copy(),
            "negi": negi,
            "c2b": c2bT,
            "c2bn": c2bnT,
        })
    return in_maps


def kernel(x, f, kernelA, u0):
    global _COMPILED
    from concourse import bass_utils

    if _COMPILED is None:
        _COMPILED = _build_program()
    nc = _COMPILED

    in_maps = _prep_core_inputs(x, f, kernelA, u0)
    res = bass_utils.run_bass_kernel_spmd(nc, in_maps, core_ids=list(range(NCORES)))
    out = np.stack([res.results[c]["out"] for c in range(NCORES)])  # [8, SPC*M, M]
    return out.reshape(B, 1, M, M).astype(np.float32)
